# revision 9
# baseline (speedup 1.0000x reference)
"""Trainium2 Bass kernel for nn_ACF_model_16707422781605 (sparse_attention).

Math note: in the reference, the item-level attention weights are produced by
``softmax(a, axis=-1)`` over a size-1 axis, which is exactly 1.0 — so
``all_a = sum_l Pi[user_pos[b, l]]`` and the whole component/item attention
stack (including every Fi gather) contributes nothing to the output. The live
computation is:

    gu  = Gu[user]                       # [B, F]
    gi  = Gi[item]                       # [B, F]
    pi  = Pi[item]                       # [B, F]
    xui = sum((gu + sum_l Pi[user_pos]) * gi, axis=1)
    return (xui, gu, gi, pi)

Sharding: pure data-parallel over the batch axis (32 users per core, no
collectives). Gu/Gi/Pi are replicated in each core's HBM; user/item/user_pos
are sharded. All gathers run on-device via indirect DMA.
"""

import sys

sys.path.insert(0, "/opt/trn_rl_repo")

import numpy as np

import concourse.bass as bass
import concourse.bacc as bacc
import concourse.mybir as mybir
from concourse.tile import TileContext
from concourse.bass_utils import run_bass_kernel_spmd

N_CORES = 8
B = 256
L = 32
F = 200
NUM_USERS = 50000
NUM_ITEMS = 10000
BC = B // N_CORES   # users per core
JW = BC * L // 128  # gathered rows per SBUF partition

_NC_CACHE = {}


def _build_nc():
    nc = bacc.Bacc("TRN2")
    f32 = mybir.dt.float32
    i32 = mybir.dt.int32

    Gu_t = nc.declare_dram_parameter("Gu", [NUM_USERS, F], f32, isOutput=False)
    Gi_t = nc.declare_dram_parameter("Gi", [NUM_ITEMS, F], f32, isOutput=False)
    Pi_t = nc.declare_dram_parameter("Pi", [NUM_ITEMS, F], f32, isOutput=False)
    user_idx = nc.declare_dram_parameter("user_idx", [BC, 1], i32, isOutput=False)
    item_idx = nc.declare_dram_parameter("item_idx", [BC, 1], i32, isOutput=False)
    pos_idx = nc.declare_dram_parameter("pos_idx", [128, JW], i32, isOutput=False)
    sel_in = nc.declare_dram_parameter("sel", [128, BC], f32, isOutput=False)
    xui_out = nc.declare_dram_parameter("xui", [BC, 1], f32, isOutput=True)
    gpp_out = nc.declare_dram_parameter("gpp_dbg", [128, JW * F], f32, isOutput=True)
    alla_out = nc.declare_dram_parameter("alla_dbg", [BC, F], f32, isOutput=True)
    gu_out = nc.declare_dram_parameter("gu", [BC, F], f32, isOutput=True)
    gi_out = nc.declare_dram_parameter("gi", [BC, F], f32, isOutput=True)
    pi_out = nc.declare_dram_parameter("pi", [BC, F], f32, isOutput=True)

    with TileContext(nc) as tc:
        with (
            tc.tile_pool(name="sb", bufs=1) as sb,
            tc.tile_pool(name="ps", bufs=1, space="PSUM") as ps,
        ):
            pos_sb = sb.tile([128, JW], i32)
            nc.sync.dma_start(out=pos_sb[:], in_=pos_idx[:, :])
            uidx_sb = sb.tile([BC, 1], i32)
            nc.sync.dma_start(out=uidx_sb[:], in_=user_idx[:, :])
            iidx_sb = sb.tile([BC, 1], i32)
            nc.sync.dma_start(out=iidx_sb[:], in_=item_idx[:, :])
            sel_sb = sb.tile([128, BC], f32)
            nc.sync.dma_start(out=sel_sb[:], in_=sel_in[:, :])

            # p_pos rows: partition p holds accesses a = p*JW + j, j in [0, JW).
            # The indirect-DMA ucode consumes exactly one index per partition,
            # so issue JW gathers of [128, F] each.
            gpp = sb.tile([128, JW * F], f32)
            for j in range(JW):
                nc.gpsimd.indirect_dma_start(
                    out=gpp[:, j * F : (j + 1) * F],
                    out_offset=None,
                    in_=Pi_t[:, :],
                    in_offset=bass.IndirectOffsetOnAxis(ap=pos_sb[:, j : j + 1], axis=0),
                )
            gu_sb = sb.tile([BC, F], f32)
            nc.gpsimd.indirect_dma_start(
                out=gu_sb[:],
                out_offset=None,
                in_=Gu_t[:, :],
                in_offset=bass.IndirectOffsetOnAxis(ap=uidx_sb[:, :1], axis=0),
            )
            gi_sb = sb.tile([BC, F], f32)
            nc.gpsimd.indirect_dma_start(
                out=gi_sb[:],
                out_offset=None,
                in_=Gi_t[:, :],
                in_offset=bass.IndirectOffsetOnAxis(ap=iidx_sb[:, :1], axis=0),
            )
            pi_sb = sb.tile([BC, F], f32)
            nc.gpsimd.indirect_dma_start(
                out=pi_sb[:],
                out_offset=None,
                in_=Pi_t[:, :],
                in_offset=bass.IndirectOffsetOnAxis(ap=iidx_sb[:, :1], axis=0),
            )

            # all_a[b, :] = sum over the 32 accesses of user b. Access a's user
            # is a // L = p // (128 // BC) for every j, so one selection matrix
            # (sel[p, b] = p//(128//BC) == b) works for all JW matmuls,
            # accumulated in PSUM.
            #
            # The PE LoadWeights slot only fits one sync wait, so let PE observe
            # the sel DMA first via a throwaway matmul; the real matmuls then
            # only need to wait on the gather.
            warm = ps.tile([BC, 1], f32)
            nc.tensor.matmul(
                out=warm[:], lhsT=sel_sb[:], rhs=sel_sb[:, 0:1], start=True, stop=True
            )
            acc = ps.tile([BC, F], f32)
            for j in range(JW):
                nc.tensor.matmul(
                    out=acc[:],
                    lhsT=sel_sb[:],
                    rhs=gpp[:, j * F : (j + 1) * F],
                    start=(j == 0),
                    stop=(j == JW - 1),
                )

            # Each DVE op may only carry one fresh sync wait: absorb the gather
            # wait with a copy, then the add only waits on the PE matmuls.
            tmp_gu = sb.tile([BC, F], f32)
            nc.vector.tensor_copy(out=tmp_gu[:], in_=gu_sb[:])
            gup = sb.tile([BC, F], f32)
            nc.vector.tensor_add(out=gup[:], in0=tmp_gu[:], in1=acc[:])
            prod = sb.tile([BC, F], f32)
            nc.vector.tensor_mul(out=prod[:], in0=gup[:], in1=gi_sb[:])
            xui_sb = sb.tile([BC, 1], f32)
            nc.vector.reduce_sum(out=xui_sb[:], in_=prod[:], axis=mybir.AxisListType.X)

            nc.sync.dma_start(out=xui_out[:, :], in_=xui_sb[:])
            nc.sync.dma_start(out=gpp_out[:, :], in_=gpp[:])
            alla_sb = sb.tile([BC, F], f32)
            nc.vector.tensor_copy(out=alla_sb[:], in_=acc[:])
            nc.sync.dma_start(out=alla_out[:, :], in_=alla_sb[:])
            nc.sync.dma_start(out=gu_out[:, :], in_=gu_sb[:])
            nc.sync.dma_start(out=gi_out[:, :], in_=gi_sb[:])
            nc.sync.dma_start(out=pi_out[:, :], in_=pi_sb[:])

    nc.finalize()
    return nc


def get_nc():
    if "nc" not in _NC_CACHE:
        _NC_CACHE["nc"] = _build_nc()
    return _NC_CACHE["nc"]


def make_in_maps(user, item, user_pos, Gu, Gi, Pi):
    user = np.ascontiguousarray(np.asarray(user).astype(np.int32))
    item = np.ascontiguousarray(np.asarray(item).astype(np.int32))
    user_pos = np.ascontiguousarray(np.asarray(user_pos).astype(np.int32))
    Gu = np.ascontiguousarray(np.asarray(Gu, dtype=np.float32))
    Gi = np.ascontiguousarray(np.asarray(Gi, dtype=np.float32))
    Pi = np.ascontiguousarray(np.asarray(Pi, dtype=np.float32))
    sel = np.ascontiguousarray(
        np.repeat(np.eye(BC, dtype=np.float32), 128 // BC, axis=0)
    )
    in_maps = []
    for k in range(N_CORES):
        s = slice(k * BC, (k + 1) * BC)
        in_maps.append(
            {
                "Gu": Gu,
                "Gi": Gi,
                "Pi": Pi,
                "user_idx": user[s].reshape(BC, 1),
                "item_idx": item[s].reshape(BC, 1),
                "pos_idx": user_pos[s].reshape(128, JW),
                "sel": sel,
            }
        )
    return in_maps


def run(user, item, user_pos, Gu, Gi, Pi, trace=False, **kw):
    nc = get_nc()
    in_maps = make_in_maps(user, item, user_pos, Gu, Gi, Pi)
    res = run_bass_kernel_spmd(
        nc, in_maps, core_ids=list(range(N_CORES)), trace=trace, **kw
    )
    xui = np.concatenate([r["xui"].reshape(BC) for r in res.results])
    gu = np.concatenate([r["gu"] for r in res.results], axis=0)
    gi = np.concatenate([r["gi"] for r in res.results], axis=0)
    pi = np.concatenate([r["pi"] for r in res.results], axis=0)
    return (xui, gu, gi, pi), res


def kernel(user, item, user_pos, Gu, Gi, Pi, **unused_weights):
    out, _ = run(user, item, user_pos, Gu, Gi, Pi)
    return out


# revision 14
# speedup vs baseline: 1.1784x; 1.1784x over previous
"""Trainium2 Bass kernel for nn_ACF_model_16707422781605 (sparse_attention),
raw Bacc (no TileContext): hand-rolled semaphores, chunked gather/compute
overlap, no Tile drain tail.

Math note: in the reference model, the item-level attention weights are
produced by ``softmax(a, axis=-1)`` over a size-1 axis, which is exactly
1.0 in floating point -- so ``all_a = sum_l Pi[user_pos[b, l]]`` and the
whole component/item attention stack (every Fi gather, cW*/iW* weights)
contributes nothing to the output. Skipping it is bit-exact dead-code
elimination. The live computation is:

    gu  = Gu[user];  gi = Gi[item];  pi = Pi[item]
    xui = sum((gu + sum_l Pi[user_pos]) * gi, axis=1)
    return (xui, gu, gi, pi)

Per core (32 users, batch-sharded, no collectives), from one combined DRAM
table T2 = [Gu; Gi; Pi]:
  - 8 indirect gathers of 128 p_pos rows (one index per SBUF partition is
    the hardware limit), each folded into the per-user PSUM accumulator by a
    selection matmul as soon as it lands;
  - 1 indirect gather of 96 rows: gi at partitions 0-31 (base 0 so the DVE
    multiply is legal; the add's other operand is PSUM, which is exempt from
    the equal-base rule), gu at 32-63, pi at 64-95;
  - DVE finishes (gu + all_a) . gi; outputs stream out as they are ready.

The 96-row gather runs second-to-last so the xui-critical tail only waits on
the last p_pos chunk.
"""

import sys

sys.path.insert(0, "/opt/trn_rl_repo")

import numpy as np

import concourse.bass as bass
import concourse.bacc as bacc
import concourse.mybir as mybir
from concourse.bass_utils import run_bass_kernel_spmd

N_CORES = 8
B = 256
L = 32
F = 200
NUM_USERS = 50000
NUM_ITEMS = 10000
PI_BASE = NUM_USERS + NUM_ITEMS
BC = B // N_CORES   # 32 users per core
JW = BC * L // 128  # 8 p_pos gather chunks

_NC_CACHE = {}


def _build_nc():
    nc = bacc.Bacc("TRN2")
    f32 = mybir.dt.float32
    i32 = mybir.dt.int32

    T2_t = nc.declare_dram_parameter(
        "T2", [NUM_USERS + 2 * NUM_ITEMS, F], f32, isOutput=False
    )
    # columns 0..JW-1: p_pos row ids (chunk j index for partition p at [p, j]);
    # column JW: rows 0-31 gi ids, 32-63 gu ids, 64-95 pi ids, rest unused
    idx_in = nc.declare_dram_parameter("idx_in", [128, JW + 1], i32, isOutput=False)
    sel_in = nc.declare_dram_parameter("sel", [128, BC], f32, isOutput=False)
    xui_out = nc.declare_dram_parameter("xui", [1, BC], f32, isOutput=True)
    gu_out = nc.declare_dram_parameter("gu", [BC, F], f32, isOutput=True)
    gi_out = nc.declare_dram_parameter("gi", [BC, F], f32, isOutput=True)
    pi_out = nc.declare_dram_parameter("pi", [BC, F], f32, isOutput=True)

    idx_sb = nc.alloc_sbuf_tensor("idx_sb", [128, JW + 1], i32)
    sel_sb = nc.alloc_sbuf_tensor("sel_sb", [128, BC], f32)
    gpp = nc.alloc_sbuf_tensor("gpp", [128, JW * F], f32)
    g3 = nc.alloc_sbuf_tensor("g3", [3 * BC, F], f32)  # gi | gu | pi
    gup = nc.alloc_sbuf_tensor("gup", [BC, F], f32)
    prod = nc.alloc_sbuf_tensor("prod", [BC, F], f32)
    xpad = nc.alloc_sbuf_tensor("xpad", [BC, BC], f32)
    xt = nc.alloc_sbuf_tensor("xt", [BC, BC], f32)
    acc = nc.alloc_psum_tensor("acc", [BC, F], f32)

    s_in = nc.alloc_semaphore("s_in")    # idx input DMA
    s_sel = nc.alloc_semaphore("s_sel")  # sel input DMA
    # one semaphore per gather op: same-queue DMA completions are unordered
    s_gs = [nc.alloc_semaphore(f"s_g{j}") for j in range(JW)]
    s_g3 = nc.alloc_semaphore("s_gx")    # combined gi/gu/pi gather
    s_pe = nc.alloc_semaphore("s_pe")
    s_v = nc.alloc_semaphore("s_v")
    s_out = nc.alloc_semaphore("s_out")

    with nc.Block() as block:

        @block.sync
        def _(sync: bass.BassEngine):
            sync.dma_start(out=idx_sb[:], in_=idx_in[:, :]).then_inc(s_in, 16)
            sync.wait_ge(s_g3, 16)
            sync.dma_start(out=gi_out[:, :], in_=g3[0:BC, :]).then_inc(s_out, 16)
            sync.dma_start(out=gu_out[:, :], in_=g3[BC : 2 * BC, :]).then_inc(
                s_out, 16
            )
            sync.dma_start(out=pi_out[:, :], in_=g3[2 * BC : 3 * BC, :]).then_inc(
                s_out, 16
            )
            sync.wait_ge(s_v, 5)
            sync.dma_start(out=xui_out[:, :], in_=xt[0:1, :]).then_inc(s_out, 16)
            sync.wait_ge(s_out, 64)

        @block.scalar
        def _(scalar: bass.BassEngine):
            scalar.dma_start(out=sel_sb[:], in_=sel_in[:, :]).then_inc(s_sel, 16)

        @block.gpsimd
        def _(gpsimd: bass.BassGpSimd):
            gpsimd.wait_ge(s_in, 16)
            for j in range(JW - 1):
                gpsimd.indirect_dma_start(
                    out=gpp[:, j * F : (j + 1) * F],
                    out_offset=None,
                    in_=T2_t[:, :],
                    in_offset=bass.IndirectOffsetOnAxis(
                        ap=idx_sb[:, j : j + 1], axis=0
                    ),
                ).then_inc(s_gs[j], 16)
            gpsimd.indirect_dma_start(
                out=g3[:, :],
                out_offset=None,
                in_=T2_t[:, :],
                in_offset=bass.IndirectOffsetOnAxis(ap=idx_sb[0 : 3 * BC, JW : JW + 1], axis=0),
            ).then_inc(s_g3, 16)
            j = JW - 1
            gpsimd.indirect_dma_start(
                out=gpp[:, j * F : (j + 1) * F],
                out_offset=None,
                in_=T2_t[:, :],
                in_offset=bass.IndirectOffsetOnAxis(ap=idx_sb[:, j : j + 1], axis=0),
            ).then_inc(s_gs[j], 16)

        @block.tensor
        def _(tensor: bass.BassEngine):
            tensor.wait_ge(s_sel, 16)
            for j in range(JW):
                tensor.wait_ge(s_gs[j], 16)
                inst = nc.tensor.matmul(
                    out=acc[:],
                    lhsT=sel_sb[:],
                    rhs=gpp[:, j * F : (j + 1) * F],
                    start=(j == 0),
                    stop=(j == JW - 1),
                )
                if j == JW - 1:
                    inst.then_inc(s_pe, 1)

        @block.vector
        def _(vector: bass.BassEngine):
            nc.vector.memset(xpad[:], 0.0).then_inc(s_v, 1)
            vector.wait_ge(s_v, 1)
            vector.wait_ge(s_g3, 16)
            vector.wait_ge(s_pe, 1)
            nc.vector.tensor_add(
                out=gup[:], in0=g3[BC : 2 * BC, :], in1=acc[:]
            ).then_inc(s_v, 1)
            vector.wait_ge(s_v, 2)
            nc.vector.tensor_mul(out=prod[:], in0=gup[:], in1=g3[0:BC, :]).then_inc(
                s_v, 1
            )
            vector.wait_ge(s_v, 3)
            nc.vector.reduce_sum(
                out=xpad[:, 0:1], in_=prod[:], axis=mybir.AxisListType.X
            ).then_inc(s_v, 1)
            vector.wait_ge(s_v, 4)
            # fold the per-partition scalars into one row: a [1, 32] store is a
            # single DMA descriptor instead of 32 4-byte ones
            nc.vector.transpose(out=xt[:, :], in_=xpad[:, :]).then_inc(s_v, 1)

    nc.finalize()
    return nc


def get_nc():
    if "nc" not in _NC_CACHE:
        _NC_CACHE["nc"] = _build_nc()
    return _NC_CACHE["nc"]


_T2_CACHE = {}


def _build_t2(Gu, Gi, Pi):
    key = id(Gu)
    if key not in _T2_CACHE:
        _T2_CACHE.clear()
        _T2_CACHE[key] = np.ascontiguousarray(
            np.concatenate([Gu, Gi, Pi], axis=0)
        )
    return _T2_CACHE[key]


def make_in_maps(user, item, user_pos, Gu, Gi, Pi):
    user = np.ascontiguousarray(np.asarray(user).astype(np.int32))
    item = np.ascontiguousarray(np.asarray(item).astype(np.int32))
    user_pos = np.ascontiguousarray(np.asarray(user_pos).astype(np.int64))
    Gu = np.asarray(Gu, dtype=np.float32)
    Gi = np.asarray(Gi, dtype=np.float32)
    Pi = np.asarray(Pi, dtype=np.float32)
    T2 = _build_t2(Gu, Gi, Pi)

    sel = np.ascontiguousarray(
        np.repeat(np.eye(BC, dtype=np.float32), 128 // BC, axis=0)
    )
    in_maps = []
    for k in range(N_CORES):
        s = slice(k * BC, (k + 1) * BC)
        idx = np.zeros((128, JW + 1), dtype=np.int32)
        idx[:, :JW] = user_pos[s].reshape(128, JW) + PI_BASE
        idx[0:BC, JW] = NUM_USERS + item[s]          # gi rows
        idx[BC : 2 * BC, JW] = user[s]               # gu rows
        idx[2 * BC : 3 * BC, JW] = PI_BASE + item[s]  # pi rows
        in_maps.append(
            {
                "T2": T2,
                "idx_in": np.ascontiguousarray(idx),
                "sel": sel,
            }
        )
    return in_maps


def run(user, item, user_pos, Gu, Gi, Pi, trace=False, **kw):
    nc = get_nc()
    in_maps = make_in_maps(user, item, user_pos, Gu, Gi, Pi)
    res = run_bass_kernel_spmd(
        nc, in_maps, core_ids=list(range(N_CORES)), trace=trace, **kw
    )
    xui = np.concatenate([r["xui"].reshape(BC) for r in res.results])
    gu = np.concatenate([r["gu"] for r in res.results], axis=0)
    gi = np.concatenate([r["gi"] for r in res.results], axis=0)
    pi = np.concatenate([r["pi"] for r in res.results], axis=0)
    return (xui, gu, gi, pi), res


def kernel(user, item, user_pos, Gu, Gi, Pi, **unused_weights):
    out, _ = run(user, item, user_pos, Gu, Gi, Pi)
    return out
